# revision 65
# baseline (speedup 1.0000x reference)
"""DTGNN Trainium2 Bass kernel (v3, latency-optimized, 15419ns vs 17149ns v2).

Single-core algorithm (graph tiny: N=8, E=16), replicated across 8 NeuronCores
via SPMD; core 0's output returned. Changes vs v2:
  - GAT1 post-normalization: aggregate ex*h unnormalized, then one fused
    relu->mul-by-1/den DVE op (relu(r*agg)=r*max(agg,0), r>0); GAT biases
    folded into h/h2 via extra contraction rows (softmax weights sum to 1).
  - CNN1 computed with swapped stationary/moving so conv outputs land
    pre-transposed (kills 2 transposes + 2 PSUM copies); conv bias via an
    extra ones row in the stationary.
  - Leaky-relus as Act Prelu(alpha=0.2) reading PSUM directly, chained
    into the Act Exp (no PSUM->SBUF copy, no cross-engine hop).
  - GAT2 attention dots (as2/ad2) computed straight off x1T with
    host-folded columns instead of a wide h2-PSUM side copy.
  - Input DMA split 4-ways: tA (consts, SP HWDGE first), tB1 (X/G1
    chunks 0-1/W17t via Pool SWDGE - skips the serialized HWDGE queue),
    tB3 (G1 chunks 2-3) + tB2 (edge attrs/MLP) on HWDGE, tC last. The h
    matmul starts on chunks 0-1 while 2-3 are still in flight.
  - cT static channels (inf/ef/er + d-biases via identity-matmul fold)
    produced early and off-chain; only the x_pool row (ch 0) remains on
    the late path, d1 bias folded via selL1's ones row.
  - CNN2 conv1 as 2x3 tap matmuls; maxpool reduces into split y1c33a/b
    tiles (no cross-engine WAW); final relu split Act/DVE.
  - Critical chain wrapped in tc.high_priority() so off-chain work
    (deconv-static, cT copies, edge MLP) never head-of-line blocks it.
Constraints honored: matmul lhsT/out partition base in {0,32,64}; engine
partition access quadrant-aligned; Pool/GPSIMD cannot touch PSUM; at most
one PSUM operand per DVE op; 8 PSUM banks via tag rotation.
"""
import numpy as np
import ml_dtypes
from contextlib import ExitStack

import concourse.bacc as bacc
import concourse.bass as bass
import concourse.tile as tile
import concourse.mybir as mybir
from concourse.bass_utils import run_bass_kernel_spmd

F32 = mybir.dt.float32
BF16 = mybir.dt.bfloat16
I32 = mybir.dt.int32
ALU = mybir.AluOpType
ACT = mybir.ActivationFunctionType
AXL = mybir.AxisListType


def _mkoff(bands):
    """name -> (col, row, nrows, ncols); blocks in one band share columns."""
    d = {}
    col = 0
    for band in bands:
        w = max(e[3] for e in band)
        for name, row, nrows, ncols in band:
            d[name] = (col, row, nrows, ncols)
        col += w
    d["_W"] = col
    return d


# tA [33, *] f32 — CNN1 + one-hot + misc consts (lands first, HWDGE)
_LA = _mkoff([
    [("T3n", 0, 25, 20)],
    [("W1se", 0, 25, 8)],
    [("w2T", 0, 10, 3)],
    [("mask16", 0, 16, 8)],
    [("M24x", 0, 16, 24)],
    [("iota_row24", 0, 24, 8)],
    [("iota8", 0, 8, 1)],
    [("ipack", 0, 24, 50)],
    [("c1b2x16", 0, 16, 1)],
    [("e17c", 0, 1, 1)],
    [("ones24", 0, 1, 24)],
    [("ident8b", 0, 8, 4)],
])
# tB1 [128, *] f32 — GAT1 h/attn weights (lands via Pool SWDGE, earliest
# big tensor); tB2 [128, *] f32 — edge attrs + edge-MLP (SP HWDGE slot 2)
_LB1 = _mkoff([
    [("XTb", 0, 128, 16)],
    [("G1bA", 0, 128, 256)],
    [("G1fb", 0, 128, 16)],
    [("W17t", 0, 17, 128)],
    [("W16asb", 0, 16, 4)],
])
_LB3 = _mkoff([
    [("G1bB", 0, 128, 256)],
])
_LB2 = _mkoff([
    [("eaT24", 0, 128, 24)],
    [("Wae", 0, 128, 4)],
    [("eaTb", 0, 128, 8)],
    [("mlpw1b", 0, 128, 32)],
    [("mlpb1", 0, 64, 1)],
    [("wv2", 0, 64, 1)],
])
# tC [128, *] f32 — GAT2 + deconv + CNN2 (lands second on HWDGE).
# Partition constraints: matmul lhsT/out bases must be 0/32/64; engine ops
# are lane-locked (in/out same partitions). cTr channels 0..3 live at
# partitions 64..67; c2w1T matches at 64..67; B4d (d2b,d3b) at 66..67 for
# the lane-aligned bias add.
_LC = _mkoff([
    [("Dst", 0, 128, 320)],
    [("D1e", 0, 65, 320)],
    [("B4dx", 64, 4, 320)],
    [("A33", 0, 33, 20), ("cTr", 64, 4, 320)],
    [("b2row", 0, 1, 33), ("c2w1T", 64, 4, 48)],
    [("eyeB", 64, 4, 2)],
    [("G2b", 0, 128, 64)],
    [("Wsd2", 0, 128, 2)],
    [("mlpw2e", 0, 65, 64)],
])
_WA = ((_LA["_W"] + 127) // 128) * 128
_WB1 = _LB1["_W"]
_WB2 = _LB2["_W"]
_WB3 = _LB3["_W"]
_WC = _LC["_W"]


def _build_nc(stage=99):
    nc = bacc.Bacc("TRN2", target_bir_lowering=False)

    mA = nc.dram_tensor("mA", [33, _WA], F32, kind="ExternalInput")
    mB1 = nc.dram_tensor("mB1", [128, _WB1], F32, kind="ExternalInput")
    mB2 = nc.dram_tensor("mB2", [128, _WB2], F32, kind="ExternalInput")
    mB3 = nc.dram_tensor("mB3", [128, _WB3], F32, kind="ExternalInput")
    mC = nc.dram_tensor("mC", [128, _WC], F32, kind="ExternalInput")
    out = nc.dram_tensor("out", [10, 64], F32, kind="ExternalOutput")

    with tile.TileContext(nc) as tc, ExitStack() as ctx:
        sb = ctx.enter_context(tc.tile_pool(name="sb", bufs=1))
        ps = ctx.enter_context(tc.tile_pool(name="ps", bufs=1, space="PSUM"))
        ctx.enter_context(nc.allow_low_precision(reason="bf16 kernel"))

        def _go():
            # ---------------------------------------------------- input DMAs
            tB1 = sb.tile([128, _WB1], F32)
            nc.gpsimd.dma_start(tB1[:], mB1[:])        # SWDGE, first on Pool
            tA = sb.tile([33, _WA], F32)
            nc.sync.dma_start(tA[:], mA[:])
            tB3 = sb.tile([128, _WB3], F32)
            nc.sync.dma_start(tB3[:], mB3[:])
            tB2 = sb.tile([128, _WB2], F32)
            nc.sync.dma_start(tB2[:], mB2[:])
            tC = sb.tile([128, _WC], F32)
            nc.sync.dma_start(tC[:], mC[:])

            def mkap(t, off, name, dt=None):
                col, row, nr, ncol = off[name]
                ap = t[row:row + nr, col:col + ncol]
                return ap.bitcast(dt) if dt is not None else ap

            def A(name, dt=None):
                return mkap(tA, _LA, name, dt)

            def Bv(name, dt=None):
                if name in _LB1:
                    return mkap(tB1, _LB1, name, dt)
                if name in _LB3:
                    return mkap(tB3, _LB3, name, dt)
                return mkap(tB2, _LB2, name, dt)

            def C(name, dt=None):
                return mkap(tC, _LC, name, dt)

            T3n = A("T3n")
            W1se = A("W1se")
            w2T = A("w2T")
            mask16 = A("mask16")
            M24x = A("M24x")
            iota_row24 = A("iota_row24")
            iota8 = A("iota8")
            c1b2x16 = A("c1b2x16")
            e17c = A("e17c")
            ones24 = A("ones24")
            ident8b = A("ident8b", BF16)

            XTb = Bv("XTb", BF16).rearrange("p (j n) -> p j n", j=4)
            G1bA = Bv("G1bA", BF16).rearrange("p (j n) -> p j n", j=2)
            G1bB = Bv("G1bB", BF16).rearrange("p (j n) -> p j n", j=2)
            G1fb = Bv("G1fb", BF16).rearrange("p (j n) -> p j n", j=4)
            W17t = Bv("W17t", BF16)
            W16asb = Bv("W16asb", BF16)
            mlpb1 = Bv("mlpb1")
            wv2 = Bv("wv2")
            eaT24 = Bv("eaT24")
            Wae = Bv("Wae")
            eaTb = Bv("eaTb", BF16)
            mlpw1b = Bv("mlpw1b", BF16)

            Dst = C("Dst", BF16)
            D1e = C("D1e", BF16)
            B4dx = C("B4dx", BF16)        # partitions 64:68: 0,inf,d2b,d3b
            cTr = C("cTr", BF16)          # partitions 64:68 = channels 0..3
            A33 = C("A33", BF16).rearrange("p (j n) -> p j n", j=4)
            c2w1T = C("c2w1T", BF16).rearrange("p (k n) -> p k n", k=3)
            b2row = C("b2row", BF16)
            eyeB = C("eyeB", BF16)        # partitions 64:68, eye(4)
            G2b = C("G2b", BF16).rearrange("p (j n) -> p j n", j=2)
            Wsd2 = C("Wsd2", BF16).rearrange("p (j s) -> p j s", j=2)
            mlpw2e = C("mlpw2e")

            # --------------------------------------------- early DVE memsets
            warm = sb.tile([1, 1], F32)
            nc.vector.memset(warm[:], 0.0)
            warm2 = sb.tile([1, 1], F32)
            nc.scalar.activation(warm2[:], warm[:], ACT.Exp)  # act table load
            zp = sb.tile([10, 24], F32)
            nc.vector.memset(zp[:], 0.0)
            x17b = sb.tile([17, 8], BF16)
            nc.vector.memset(x17b[:], 1.0)    # row 16 stays 1.0 (bias row)
            s2 = sb.tile([65, 2], F32)
            nc.vector.memset(s2[64:65, :], 1.0)
            y1c33a = sb.tile([33, 32, 4], BF16)
            nc.vector.memset(y1c33a[32:33, :, :], 1.0)
            y1c33b = sb.tile([33, 32, 4], BF16)
            nc.vector.memset(y1c33b[32:33, :, :], 1.0)
            selL1 = sb.tile([65, 1], BF16)
            nc.vector.memset(selL1[64:65, :], 1.0)
            sel128v = sb.tile([128, 4], BF16)
            nc.vector.memset(sel128v[:], 0.0)
            ones1b = sb.tile([1, 8], BF16)
            nc.vector.memset(ones1b[:], 1.0)
            ones8b = sb.tile([8, 1], BF16)
            nc.vector.memset(ones8b[:], 0.125)

            # --------------------------------------------- one-hot matrices
            ti = A("ipack").bitcast(I32)
            tif = sb.tile([24, 50], F32)
            idx_f = tif[0:8, 0:48].rearrange("p (c e) -> p c e", c=2)
            dcol_f = tif[:, 48:49]
            PsrcTf = sb.tile([8, 24], F32)
            PsrcTb = sb.tile([8, 24], BF16)
            PdstTf = sb.tile([8, 24], F32)
            Pdst24f = sb.tile([24, 8], F32)
            Pdst24b = sb.tile([24, 8], BF16)

            # =========================================================== CNN1
            ps_y1T = ps.tile([10, 16], F32, tag="sm", bufs=2)
            nc.tensor.matmul(ps_y1T[:, 0:8], T3n[:, 0:10], W1se,
                             start=True, stop=True)
            nc.tensor.matmul(ps_y1T[:, 8:16], T3n[:, 10:20], W1se,
                             start=True, stop=True)
            zpv = zp[:].rearrange("p (t v) -> p t v", v=2)
            nc.vector.tensor_scalar(
                zpv[:, 1:9, :],
                ps_y1T[:].rearrange("p (b n) -> p n b", b=2),
                0.0, None, ALU.max)

            ps_y2T = ps.tile([16, 1], F32, tag="sm", bufs=2)
            for k in range(3):
                nc.tensor.matmul(ps_y2T[:], zp[:, 2 * k:2 * k + 16],
                                 w2T[:, k:k + 1],
                                 start=(k == 0), stop=(k == 2))
            xv16 = sb.tile([16, 1], F32)
            nc.vector.tensor_scalar(xv16[:], ps_y2T[:], c1b2x16, 0.0,
                                    ALU.add, ALU.max)
            # bf16 x16 (shared by h and attention paths)
            nc.vector.tensor_tensor(x17b[0:16, :],
                                    xv16[:].broadcast_to([16, 8]),
                                    mask16, ALU.mult)

            # one-hot builds (DVE, slack before GAT1 needs them)
            nc.vector.tensor_copy(tif[:], ti)
            nc.vector.tensor_scalar(PsrcTf[:], idx_f[:, 0, :], iota8, None,
                                    ALU.is_equal)
            nc.vector.tensor_scalar(PsrcTb[:], idx_f[:, 0, :], iota8, None,
                                    ALU.is_equal)
            nc.vector.tensor_scalar(PdstTf[:], idx_f[:, 1, :], iota8, None,
                                    ALU.is_equal)
            nc.vector.tensor_scalar(Pdst24f[:], iota_row24, dcol_f, None,
                                    ALU.is_equal)
            nc.vector.tensor_scalar(Pdst24b[:], iota_row24, dcol_f, None,
                                    ALU.is_equal)
            if stage == 1:
                o10 = sb.tile([10, 64], F32)
                nc.vector.memset(o10[:], 0.0)
                nc.vector.tensor_copy(o10[0:10, 0:24], zp[:])
                nc.vector.tensor_copy(o10[0:10, 30:31], xv16[0:10, :])
                nc.sync.dma_start(out[:], o10[:])
                return

            # ================================================ GAT1 h + attn
            with tc.high_priority():
                # attention projections (bf16): as/ad folded columns. The
                # x16 chunk goes first so the static scheduler orders this
                # group after CNN1's tiny matmuls (avoids PE head-of-line).
                ps_hf = ps.tile([8, 8], F32, tag="sm", bufs=2)
                nc.tensor.matmul(ps_hf[:], x17b[0:16, :], W16asb,
                                 start=True, stop=False)
                for j in range(4):
                    nc.tensor.matmul(ps_hf[:], XTb[:, j, :], G1fb[:, j, :],
                                     start=False, stop=(j == 3))
                # h (bf16) with g1_b folded via x17b row 16 x W17t row 16
                ps_h = ps.tile([8, 256], F32, tag="big", bufs=2)
                nc.tensor.matmul(ps_h[:], x17b[:], W17t, start=True,
                                 stop=False)
                for j in range(2):
                    nc.tensor.matmul(ps_h[:], XTb[:, j, :], G1bA[:, j, :],
                                     start=False, stop=False)
                for j in range(2):
                    nc.tensor.matmul(ps_h[:], XTb[:, 2 + j, :], G1bB[:, j, :],
                                     start=False, stop=(j == 1))

                asad = sb.tile([8, 8], F32)
                nc.vector.tensor_copy(asad[:], ps_hf[:])
                # h' -> SBUF (DVE; Pool can't touch PSUM, Act has the alphas)
                h_sb = sb.tile([8, 256], BF16)
                nc.vector.tensor_copy(h_sb[:], ps_h[:])

                # alpha1 = ea@(We@ae) + (h.as)[src] + (h.ad)[dst]
                ps_al = ps.tile([24, 4], F32, tag="sm", bufs=2)
                nc.tensor.matmul(ps_al[:], eaT24, Wae, start=True, stop=False)
                nc.tensor.matmul(ps_al[:], PsrcTf[:], asad[:, 0:4],
                                 start=False, stop=False)
                nc.tensor.matmul(ps_al[:], PdstTf[:], asad[:, 4:8],
                                 start=False, stop=True)
                lr1 = sb.tile([24, 4], F32)
                nc.scalar.activation(lr1[:], ps_al[:], ACT.Prelu, alpha=0.2)
                ex24 = sb.tile([24, 4], F32)
                nc.scalar.activation(ex24[:], lr1[:], ACT.Exp)

                # gather h'[src] (PE) once h' in SBUF
                ps_sg = ps.tile([24, 256], F32, tag="big", bufs=2)
                nc.tensor.matmul(ps_sg[:], PsrcTb[:], h_sb[:], start=True,
                                 stop=True)
                ps_den = ps.tile([8, 4], F32, tag="sm", bufs=2)
                nc.tensor.matmul(ps_den[:], Pdst24f[:], ex24[:], start=True,
                                 stop=True)
                rden = sb.tile([8, 4], F32)
                nc.vector.reciprocal(rden[:], ps_den[:])

            # edge MLP (normal priority: fills PE/DVE idle slots)
            ps_m1 = ps.tile([64, 16], F32, tag="sm", bufs=2)
            nc.tensor.matmul(ps_m1[:], mlpw1b, eaTb, start=True, stop=True)
            r1T = sb.tile([64, 16], F32)
            nc.vector.tensor_scalar(r1T[:], ps_m1[:], mlpb1, 0.0, ALU.add,
                                    ALU.max)
            # e16 (edge attn vec for GAT2), off critical chain
            ps_e16 = ps.tile([16, 1], F32, tag="sm", bufs=2)
            nc.tensor.matmul(ps_e16[:], r1T[:], wv2, start=True, stop=True)
            e16sb = sb.tile([16, 1], F32)
            nc.scalar.copy(e16sb[:], ps_e16[:])
            if stage == 22:
                o10 = sb.tile([10, 64], F32)
                nc.vector.memset(o10[:], 0.0)
                nc.vector.tensor_copy(o10[0:10, 0:4], lr1[0:10, :])
                nc.vector.tensor_copy(o10[0:10, 4:8], ex24[0:10, :])
                nc.vector.tensor_copy(o10[0:8, 8:12], rden[:])
                nc.vector.tensor_copy(o10[0:10, 12:16], ps_al[0:10, :])
                nc.sync.dma_start(out[:], o10[:])
                return

            if stage == 21:
                o10 = sb.tile([10, 64], F32)
                nc.vector.memset(o10[:], 0.0)
                nc.vector.tensor_copy(o10[0:8, 0:56], ps_h[:, 0:56])
                nc.vector.tensor_copy(o10[0:8, 56:64], asad[:])
                nc.sync.dma_start(out[:], o10[:])
                return

            # unnormalized weighted gather + post-normalized relu
            with tc.high_priority():
                wh24 = sb.tile([24, 256], BF16)
                nc.vector.tensor_tensor(
                    wh24[:].rearrange("p (h c) -> p h c", h=4),
                    ps_sg[:].rearrange("p (h c) -> p h c", h=4),
                    ex24[:].broadcast_to([24, 4, 64]), ALU.mult)
                ps_x1 = ps.tile([8, 256], F32, tag="big", bufs=2)
                nc.tensor.matmul(ps_x1[:], Pdst24b[:], wh24[:], start=True,
                                 stop=True)
                x1 = sb.tile([8, 256], BF16)
                nc.vector.scalar_tensor_tensor(
                    x1[:].rearrange("p (h c) -> p h c", h=4),
                    ps_x1[:].rearrange("p (h c) -> p h c", h=4),
                    0.0, rden[:].broadcast_to([8, 4, 64]), ALU.max, ALU.mult)
            # edge sums for ef/er deconv (off-chain; after critical DVE ops)
            r1v = r1T[:].rearrange("p (e two) -> p two e", two=2)
            nc.vector.tensor_reduce(s2[0:64, 0:1], r1v[:, 0, :], axis=AXL.X,
                                    op=ALU.add)
            nc.vector.tensor_reduce(s2[0:64, 1:2], r1v[:, 1, :], axis=AXL.X,
                                    op=ALU.add)
            if stage == 2:
                o10 = sb.tile([10, 64], F32)
                nc.vector.memset(o10[:], 0.0)
                nc.vector.tensor_copy(o10[0:8, 0:32],
                                      x1[:, 0:64].bitcast(F32))
                nc.sync.dma_start(out[:], o10[:])
                return

            # ---------------- deconv static channels (ef/er), off-chain ----
            ps_sel = ps.tile([128, 1], F32, tag="sm", bufs=2)
            nc.tensor.matmul(ps_sel[0:64, 0:1], mlpw2e, s2[:, 0:1],
                             start=True, stop=True)
            nc.tensor.matmul(ps_sel[64:128, 0:1], mlpw2e, s2[:, 1:2],
                             start=True, stop=True)
            nc.scalar.copy(sel128v[0:64, 2:3], ps_sel[0:64, 0:1])
            nc.scalar.copy(sel128v[64:128, 3:4], ps_sel[64:128, 0:1])

            # [inf, d2b, d3b] const rows folded in via identity matmul, so
            # the SBUF move is a plain copy; ch0 comes out zero here and is
            # overwritten by the late x_pool copy.
            ps_cSa = ps.tile([68, 320], F32, tag="ct", bufs=2)
            nc.tensor.matmul(ps_cSa[64:68, :], sel128v[:], Dst[:, 0:320],
                             start=True, stop=False)
            nc.tensor.matmul(ps_cSa[64:68, :], eyeB[:, 0:4], B4dx[:, 0:320],
                             start=False, stop=True)
            ps_cSb = ps.tile([68, 320], F32, tag="ct", bufs=2)
            nc.tensor.matmul(ps_cSb[64:68, :], sel128v[:], Dst[:, 320:640],
                             start=True, stop=False)
            nc.tensor.matmul(ps_cSb[64:68, :], eyeB[:, 0:4], B4dx[:, 320:640],
                             start=False, stop=True)
            nc.vector.tensor_copy(cTr[0:4, 0:320], ps_cSa[64:68, :])
            # (the b-half copy is emitted after GAT2 so it lands in Act's
            # idle window after ex2 rather than blocking the alpha2 path)

            # ======================================================== GAT2
            with tc.high_priority():
                ps_tr = ps.tile([128, 16], BF16, tag="sm", bufs=2)
                nc.tensor.transpose(ps_tr[:, 0:8], x1[:, 0:128], ident8b)
                nc.tensor.transpose(ps_tr[:, 8:16], x1[:, 128:256], ident8b)
                x1T = sb.tile([128, 2, 8], BF16)
                nc.vector.tensor_copy(
                    x1T[:].rearrange("p j n -> p (j n)"), ps_tr[:])

                # as2/ad2 attention dots and h2 (bf16, g2_b folded), both
                # straight off x1T — no wide-PSUM side copy on the alpha path
                ps_as2 = ps.tile([8, 2], F32, tag="sm", bufs=2)
                for j in range(2):
                    nc.tensor.matmul(ps_as2[:], x1T[:, j, :], Wsd2[:, j, :],
                                     start=(j == 0), stop=(j == 1))
                ps_h2 = ps.tile([8, 64], F32, tag="sm", bufs=2)
                nc.tensor.matmul(ps_h2[:], ones1b[:], b2row[:, 0:64],
                                 start=True, stop=False)
                for j in range(2):
                    nc.tensor.matmul(ps_h2[:], x1T[:, j, :], G2b[:, j, :],
                                     start=False, stop=(j == 1))
                asad2 = sb.tile([8, 2], F32)
                nc.vector.tensor_copy(asad2[:], ps_as2[:])
                hs2b = sb.tile([8, 64], BF16)
                nc.scalar.copy(hs2b[:], ps_h2[:])

                ps_al2 = ps.tile([24, 1], F32, tag="sm", bufs=2)
                nc.tensor.matmul(ps_al2[:], ones24, e17c, start=True,
                                 stop=False)
                nc.tensor.matmul(ps_al2[:], M24x, e16sb[:], start=False,
                                 stop=False)
                nc.tensor.matmul(ps_al2[:], PsrcTf[:], asad2[:, 0:1],
                                 start=False, stop=False)
                nc.tensor.matmul(ps_al2[:], PdstTf[:], asad2[:, 1:2],
                                 start=False, stop=True)
                ps_sg2 = ps.tile([24, 64], F32, tag="sm", bufs=2)
                nc.tensor.matmul(ps_sg2[:], PsrcTb[:], hs2b[:], start=True,
                                 stop=True)
                lr2 = sb.tile([24, 1], F32)
                nc.scalar.activation(lr2[:], ps_al2[:], ACT.Prelu, alpha=0.2)
                ex2 = sb.tile([24, 1], F32)
                nc.scalar.activation(ex2[:], lr2[:], ACT.Exp)
                ps_den2 = ps.tile([8, 1], F32, tag="sm", bufs=2)
                nc.tensor.matmul(ps_den2[:], Pdst24f[:], ex2[:], start=True,
                                 stop=True)
                rden2 = sb.tile([8, 1], F32)
                nc.vector.reciprocal(rden2[:], ps_den2[:])
                wh2 = sb.tile([24, 64], BF16)
                nc.vector.tensor_scalar(wh2[:], ps_sg2[:], ex2[:], None,
                                        ALU.mult)
                ps_x2u = ps.tile([8, 64], F32, tag="sm", bufs=2)
                nc.tensor.matmul(ps_x2u[:], Pdst24b[:], wh2[:], start=True,
                                 stop=True)
                x2 = sb.tile([8, 64], BF16)
                nc.vector.tensor_scalar(x2[:], ps_x2u[:], rden2[:], 0.0,
                                        ALU.mult, ALU.max)
            nc.scalar.copy(cTr[0:4, 320:640], ps_cSb[64:68, :])
            if stage == 3:
                o10 = sb.tile([10, 64], F32)
                nc.vector.memset(o10[:], 0.0)
                nc.vector.tensor_copy(o10[0:8, 0:32], x2[:].bitcast(F32))
                nc.sync.dma_start(out[:], o10[:])
                return

            # ------------------------------- late path: x_pool row (ch 0) --
            with tc.high_priority():
                ps_xm = ps.tile([64, 1], F32, tag="sm", bufs=2)
                nc.tensor.matmul(ps_xm[:], x2[:], ones8b[:], start=True,
                                 stop=True)
                nc.vector.tensor_copy(selL1[0:64, :], ps_xm[:])
                ps_c0a = ps.tile([65, 320], F32, tag="c0a", bufs=1)
                nc.tensor.matmul(ps_c0a[64:65, :], selL1[:], D1e[:, 0:320],
                                 start=True, stop=True)
                ps_c0b = ps.tile([65, 320], F32, tag="c0b", bufs=1)
                nc.tensor.matmul(ps_c0b[64:65, :], selL1[:], D1e[:, 320:640],
                                 start=True, stop=True)
                nc.vector.tensor_copy(cTr[0:1, 0:320], ps_c0a[64:65, :])
                nc.scalar.copy(cTr[0:1, 320:640], ps_c0b[64:65, :])
            if stage == 4:
                o10 = sb.tile([10, 64], F32)
                nc.vector.memset(o10[:], 0.0)
                nc.sync.dma_start(out[0:4, 0:40],
                                  mkap(tC, _LC, "cTr")[:, 0:40])
                nc.sync.dma_start(out[4:10, :], o10[4:10, :])
                return

            # ======================================================== CNN2
            with tc.high_priority():
                cTv = cTr[:].rearrange("p (b l) -> p b l", b=64)
                ps_c1a = ps.tile([32, 32, 8], F32, tag="ct", bufs=2)
                for k in range(3):
                    nc.tensor.matmul(ps_c1a[:], c2w1T[:, k, :],
                                     cTv[:, 0:32, k:k + 8],
                                     start=(k == 0), stop=(k == 2))
                ps_c1b = ps.tile([32, 32, 8], F32, tag="ct", bufs=2)
                for k in range(3):
                    nc.tensor.matmul(ps_c1b[:], c2w1T[:, k, :],
                                     cTv[:, 32:64, k:k + 8],
                                     start=(k == 0), stop=(k == 2))
                nc.vector.tensor_reduce(
                    y1c33a[0:32, :, :],
                    ps_c1a[:].rearrange("p b (l two) -> p b l two", two=2),
                    axis=AXL.X, op=ALU.max)
                nc.vector.tensor_reduce(
                    y1c33b[0:32, :, :],
                    ps_c1b[:].rearrange("p b (l two) -> p b l two", two=2),
                    axis=AXL.X, op=ALU.max)

                o10 = sb.tile([10, 64], F32)
                ps_outA = ps.tile([10, 32], F32, tag="sm", bufs=2)
                for j in range(4):
                    nc.tensor.matmul(ps_outA[:], A33[:, j, :],
                                     y1c33a[:, :, j],
                                     start=(j == 0), stop=(j == 3))
                nc.scalar.activation(o10[:, 0:32], ps_outA[:], ACT.Relu)
                ps_outB = ps.tile([10, 32], F32, tag="sm", bufs=2)
                for j in range(4):
                    nc.tensor.matmul(ps_outB[:], A33[:, j, :],
                                     y1c33b[:, :, j],
                                     start=(j == 0), stop=(j == 3))
                nc.vector.tensor_scalar(o10[:, 32:64], ps_outB[:], 0.0,
                                        None, ALU.max)
                nc.sync.dma_start(out[:], o10[:])
                return

            # ------------------------------- late path: x_pool row (ch 0) --
            ps_xm = ps.tile([64, 1], F32, tag="sm", bufs=2)
            nc.tensor.matmul(ps_xm[:], x2[:], ones8b[:], start=True,
                             stop=True)
            nc.vector.tensor_copy(selL1[0:64, :], ps_xm[:])
            ps_c0a = ps.tile([65, 320], F32, tag="c0a", bufs=1)
            nc.tensor.matmul(ps_c0a[64:65, :], selL1[:], D1e[:, 0:320],
                             start=True, stop=True)
            ps_c0b = ps.tile([65, 320], F32, tag="c0b", bufs=1)
            nc.tensor.matmul(ps_c0b[64:65, :], selL1[:], D1e[:, 320:640],
                             start=True, stop=True)
            nc.vector.tensor_copy(cTr[0:1, 0:320], ps_c0a[64:65, :])
            nc.scalar.copy(cTr[0:1, 320:640], ps_c0b[64:65, :])
            if stage == 4:
                o10 = sb.tile([10, 64], F32)
                nc.vector.memset(o10[:], 0.0)
                nc.sync.dma_start(out[0:4, 0:40],
                                  mkap(tC, _LC, "cTr")[:, 0:40])
                nc.sync.dma_start(out[4:10, :], o10[4:10, :])
                return

            # ======================================================== CNN2
            cTv = cTr[:].rearrange("p (b l) -> p b l", b=64)
            ps_c1a = ps.tile([32, 32, 8], F32, tag="ct", bufs=2)
            for k in range(3):
                nc.tensor.matmul(ps_c1a[:], c2w1T[:, k, :],
                                 cTv[:, 0:32, k:k + 8],
                                 start=(k == 0), stop=(k == 2))
            ps_c1b = ps.tile([32, 32, 8], F32, tag="ct", bufs=2)
            for k in range(3):
                nc.tensor.matmul(ps_c1b[:], c2w1T[:, k, :],
                                 cTv[:, 32:64, k:k + 8],
                                 start=(k == 0), stop=(k == 2))
            nc.vector.tensor_reduce(
                y1c33a[0:32, :, :],
                ps_c1a[:].rearrange("p b (l two) -> p b l two", two=2),
                axis=AXL.X, op=ALU.max)
            nc.vector.tensor_reduce(
                y1c33b[0:32, :, :],
                ps_c1b[:].rearrange("p b (l two) -> p b l two", two=2),
                axis=AXL.X, op=ALU.max)

            o10 = sb.tile([10, 64], F32)
            ps_outA = ps.tile([10, 32], F32, tag="sm", bufs=2)
            for j in range(4):
                nc.tensor.matmul(ps_outA[:], A33[:, j, :], y1c33a[:, :, j],
                                 start=(j == 0), stop=(j == 3))
            nc.scalar.activation(o10[:, 0:32], ps_outA[:], ACT.Relu)
            ps_outB = ps.tile([10, 32], F32, tag="sm", bufs=2)
            for j in range(4):
                nc.tensor.matmul(ps_outB[:], A33[:, j, :], y1c33b[:, :, j],
                                 start=(j == 0), stop=(j == 3))
            nc.vector.tensor_scalar(o10[:, 32:64], ps_outB[:], 0.0,
                                    None, ALU.max)
            nc.sync.dma_start(out[:], o10[:])

        _go()
    nc.finalize()
    return nc


_NC = None


def _get_nc():
    global _NC
    if _NC is None:
        _NC = _build_nc()
    return _NC


def _bfpack(a):
    """[r, c] float array -> [r, c/2] f32 whose bits hold bf16 pairs."""
    a = np.ascontiguousarray(np.asarray(a, dtype=np.float32))
    r, c = a.shape
    assert c % 2 == 0, c
    u = a.astype(ml_dtypes.bfloat16).view(np.uint16).reshape(r, c // 2, 2)
    packed = u[:, :, 0].astype(np.uint32) | (u[:, :, 1].astype(np.uint32) << 16)
    return packed.view(np.float32)


def _pack_inputs(x_feat, x_feat_tmp, edge_attr, c1w1, c1b1, c1w2, c1b2,
                 g1_lin, g1_as, g1_ad, g1_le, g1_ae, g1_b,
                 g2_lin, g2_as, g2_ad, g2_le, g2_ae, g2_b,
                 mlp_w1, mlp_b1, mlp_w2, mlp_b2,
                 d1w, d1b, d2w, d2b, d3w, d3b,
                 c2w1, c2b1, c2w2, c2b2, c2l1w, c2l1b, c2l2w, c2l2b,
                 edge_index):
    f = np.float32
    x_feat = np.asarray(x_feat, f)
    x_feat_tmp = np.asarray(x_feat_tmp, f)
    edge_attr = np.asarray(edge_attr, f)

    def fill(shape, off, blocks):
        arr = np.zeros(shape, dtype=f)
        for name, a in blocks.items():
            a = np.asarray(a, dtype=f)
            col, row, nr, ncol = off[name]
            assert a.shape[0] <= nr and a.shape[1] <= ncol, (name, a.shape)
            arr[row:row + a.shape[0], col:col + a.shape[1]] = a
        return arr

    # ---- tA ----
    tpad = np.zeros((8, 2, 12), dtype=f)
    for i in range(8):
        r = 1 if i % 2 == 0 else 5
        for b in range(2):
            tpad[i, b, 1:11] = x_feat_tmp[r, b * 4 + i // 2]
    T3n = np.zeros((25, 20), dtype=f)
    for k in range(3):
        for c in range(8):
            for b in range(2):
                T3n[k * 8 + c, b * 10:(b + 1) * 10] = tpad[c, b, k:k + 10]
    T3n[24, :] = 1.0
    W1se = np.zeros((25, 8), dtype=f)
    W1se[0:24] = np.asarray(c1w1, f).transpose(2, 1, 0).reshape(24, 8)
    W1se[24] = np.asarray(c1b1, f)

    mask16 = np.zeros((16, 8), dtype=f)
    for c in range(2):
        for n in range(8):
            mask16[c * 8 + n, n] = 1.0
    f16 = np.array([(j % 2) * 8 + j // 2 for j in range(16)])
    perm16 = np.array([(v % 2) * 8 + v // 2 for v in f16])
    mask16 = mask16[perm16]

    M24x = np.zeros((16, 24), dtype=f)
    M24x[0:16, 0:16] = np.eye(16, dtype=f)
    M24x[0:16, 16:24] = 1.0 / 16.0

    ve2 = np.asarray(g2_le, f) @ np.asarray(g2_ae, f).reshape(64)  # [64]
    wv2 = (np.asarray(mlp_w2, f) @ ve2).reshape(64, 1)
    e17c = float(np.asarray(mlp_b2, f) @ ve2)

    ipack = np.zeros((24, 50), dtype=np.int32)
    blk = np.zeros((8, 2, 24), dtype=np.int32)
    blk[:, :, 0:16] = np.asarray(edge_index, np.int32)[None, :, :]
    blk[:, :, 16:24] = np.arange(8, dtype=np.int32)[None, None, :]
    ipack[0:8, 0:48] = blk.reshape(8, 48)
    ipack[0:16, 48] = np.asarray(edge_index, np.int32)[1]
    ipack[16:24, 48] = np.arange(8, dtype=np.int32)

    tAm = fill((33, _WA), _LA, {
        "T3n": T3n, "W1se": W1se,
        "w2T": np.asarray(c1w2, f).transpose(1, 2, 0).reshape(10, 3),
        "mask16": mask16, "M24x": M24x,
        "iota_row24": np.broadcast_to(np.arange(8, dtype=f), (24, 8)),
        "iota8": np.arange(8, dtype=f).reshape(8, 1),
        "ipack": ipack.view(np.float32),
        "c1b2x16": np.broadcast_to(np.asarray(c1b2, f).reshape(1, 1),
                                   (16, 1)),
        "e17c": np.array([[e17c]], dtype=f),
        "ones24": np.ones((1, 24), dtype=f),
        "ident8b": _bfpack(np.eye(8, dtype=f)),
    })

    # ---- tB ----
    xfT = np.zeros((512, 8), dtype=f)
    xfT[0:510] = x_feat.T
    XT4 = xfT.reshape(4, 128, 8).transpose(1, 0, 2).reshape(128, 32)
    G1 = np.asarray(g1_lin, f)                       # [512, 256]
    Was = np.zeros((512, 4), dtype=f)
    Wad = np.zeros((512, 4), dtype=f)
    a_s = np.asarray(g1_as, f)
    a_d = np.asarray(g1_ad, f)
    for h in range(4):
        Was[:, h] = G1[:, h * 64:(h + 1) * 64] @ a_s[h]
        Wad[:, h] = G1[:, h * 64:(h + 1) * 64] @ a_d[h]
    G1f8 = np.concatenate([Was, Wad], 1)             # [512, 8]
    Wae = (np.asarray(g1_le, f).reshape(128, 4, 64) *
           np.asarray(g1_ae, f)[None]).sum(-1)       # [128, 4]
    ea_mean = edge_attr.mean(0)
    ea24 = np.concatenate([edge_attr, np.broadcast_to(ea_mean, (8, 128))], 0)

    W17 = np.zeros((17, 256), dtype=f)
    W17[0:16] = np.repeat(G1[510:512], 8, axis=0)[perm16]
    W17[16] = np.asarray(g1_b, f)

    G1p = G1.reshape(4, 128, 256).transpose(1, 0, 2)      # [128, 4, 256]
    tB1m = fill((128, _WB1), _LB1, {
        "XTb": _bfpack(XT4),
        "G1bA": _bfpack(G1p[:, 0:2].reshape(128, 512)),
        "G1fb": _bfpack(
            G1f8.reshape(4, 128, 8).transpose(1, 0, 2).reshape(128, 32)),
        "W17t": _bfpack(W17),
        "W16asb": _bfpack(np.repeat(G1f8[510:512], 8, axis=0)[perm16]),
    })
    tB3m = fill((128, _WB3), _LB3, {
        "G1bB": _bfpack(G1p[:, 2:4].reshape(128, 512)),
    })
    tB2m = fill((128, _WB2), _LB2, {
        "eaT24": ea24.T,
        "Wae": Wae,
        "eaTb": _bfpack(edge_attr.T),
        "mlpw1b": _bfpack(np.asarray(mlp_w1, f)),
        "mlpb1": np.asarray(mlp_b1, f).reshape(64, 1),
        "wv2": wv2,
    })

    # ---- tC ----
    G2 = np.asarray(g2_lin, f)                        # [256, 64]
    was2 = (G2 @ np.asarray(g2_as, f).reshape(64)).reshape(256, 1)
    wad2 = (G2 @ np.asarray(g2_ad, f).reshape(64)).reshape(256, 1)
    G2e4 = G2.reshape(2, 128, 64).transpose(1, 0, 2).reshape(128, 128)
    Wsd2 = np.concatenate([was2, wad2], 1)            # [256, 2]
    Wsd2 = Wsd2.reshape(2, 128, 2).transpose(1, 0, 2).reshape(128, 4)

    mlpw2e = np.zeros((65, 64), dtype=f)
    mlpw2e[0:64] = np.asarray(mlp_w2, f) * 0.125
    mlpw2e[64] = np.asarray(mlp_b2, f)

    Dst = np.concatenate([np.asarray(d2w, f).reshape(64, 640),
                          np.asarray(d3w, f).reshape(64, 640)], 0)
    D1e = np.zeros((65, 640), dtype=f)
    D1e[0:64] = np.asarray(d1w, f).reshape(64, 640)
    D1e[64] = np.repeat(np.asarray(d1b, f), 10)
    B4dx = np.zeros((4, 640), dtype=f)  # rows: 0, inf, d2b-rep, d3b-rep
    B4dx[1] = x_feat_tmp.reshape(640)
    B4dx[2] = np.repeat(np.asarray(d2b, f), 10)
    B4dx[3] = np.repeat(np.asarray(d3b, f), 10)

    b2e = np.zeros((1, 66), dtype=f)
    b2e[0, 0:64] = np.asarray(g2_b, f)

    # CNN2 fold: Wf [128,10] over (c2, l2); A[(c1,j), t]; const row.
    Wf = np.asarray(c2l1w, f) @ np.asarray(c2l2w, f)            # [128, 10]
    bfold = np.asarray(c2l1b, f) @ np.asarray(c2l2w, f) + np.asarray(c2l2b, f)
    WfR = Wf.reshape(64, 2, 10)                                 # [c2, l2, t]
    w2c = np.asarray(c2w2, f)                                   # [64, 32, 3]
    Afold = np.zeros((32, 4, 10), dtype=f)                      # [c1, j, t]
    for j in range(4):
        for l2 in range(2):
            k = j - l2
            if 0 <= k <= 2:
                Afold[:, j, :] += np.einsum("co,ct->ot", w2c[:, :, k],
                                            WfR[:, l2, :])
    const = (np.einsum("c,clt->t",
                       np.asarray(c2b2, f), WfR) + bfold +
             np.einsum("c,cjt->t", np.asarray(c2b1, f), Afold))
    A33m = np.zeros((33, 40), dtype=f)
    A33m[0:32] = Afold.reshape(32, 40)
    A33m[32, 0:10] = const
    tCm = fill((128, _WC), _LC, {
        "Dst": _bfpack(Dst),
        "D1e": _bfpack(D1e),
        "B4dx": _bfpack(B4dx),
        "b2row": _bfpack(b2e),
        "A33": _bfpack(A33m),
        "c2w1T": _bfpack(np.asarray(c2w1, f).transpose(1, 2, 0).reshape(4, 96)),
        "G2b": _bfpack(G2e4),
        "Wsd2": _bfpack(Wsd2),
        "mlpw2e": mlpw2e,
        "eyeB": _bfpack(np.eye(4, dtype=f)),
    })
    return tAm, tB1m, tB2m, tB3m, tCm


def _make_ins(inputs):
    tAm, tB1m, tB2m, tB3m, tCm = _pack_inputs(**inputs)
    return {"mA": tAm, "mB1": tB1m, "mB2": tB2m, "mB3": tB3m, "mC": tCm}


def kernel(**inputs):
    inputs = {k: np.ascontiguousarray(v) for k, v in inputs.items()}
    ins = _make_ins(inputs)
    nc = _get_nc()
    res = run_bass_kernel_spmd(nc, [ins] * 8, core_ids=list(range(8)))
    return np.ascontiguousarray(res.results[0]["out"].T).reshape(8, 8, 10)
